# revision 10
# baseline (speedup 1.0000x reference)
"""ContextBasedLinear Trainium2 kernel.

Computes out = mu * x + gamma * sum(x, axis=1, keepdims=True) for
x: [64, 1024, 512] f32, mu/gamma: [1] f32.

Sharding: data-parallel on the batch dim — 8 batches per core on 8
NeuronCores; mu/gamma replicated. No cross-core comms needed.

Per-core program (x_c: [8, 1024, 512]):
  Each batch's [1024, 512] lives in SBUF as [128, 4096]: partition p
  holds set rows 8p..8p+7 (16 KB contiguous per partition), processed
  in two half-tiles [128, 2048] for pipelining.
  - colsum: PE matmuls with ones[128,1] stationary reduce the
    partition dim of each 512-wide r-slice, accumulating all 8 slices
    into one PSUM row psum_s[1, 512].
  - psum_b[128,512] = (gamma ones)[1,128].T @ s[1,512]: rank-1 matmul
    broadcasts gamma * colsum to every partition.
  - out = (x * mu) + psum_b in ONE fused DVE scalar_tensor_tensor pass
    per half, with psum_b read through a step-0 broadcast AP.
  - loads issue on the SP HWDGE ring (nc.sync), stores on the ACT ring
    (nc.scalar) so store-waits can't head-of-line-block loads.
"""

import numpy as np

import concourse.bacc as bacc
import concourse.mybir as mybir
import concourse.tile as tile
from concourse.bass_utils import run_bass_kernel_spmd

N_CORES = 8
B_FULL = 64
B_PER = B_FULL // N_CORES  # 8 batches per core
N_SET = 1024
D = 512
P = 128
R = N_SET // P  # 8 set-rows per partition
F = R * D  # 4096 free elems per partition
H = 2  # half-tiles per batch
RH = R // H  # 4 r-slices per half
FH = F // H  # 2048 free elems per half

_cache = {}


def build_nc():
    if "nc" in _cache:
        return _cache["nc"]
    f32 = mybir.dt.float32
    nc = bacc.Bacc(
        "TRN2", target_bir_lowering=False, debug=False, num_devices=N_CORES
    )
    x_d = nc.dram_tensor("x", [B_PER, N_SET, D], f32, kind="ExternalInput").ap()
    mu_d = nc.dram_tensor("mu", [1], f32, kind="ExternalInput").ap()
    gamma_d = nc.dram_tensor("gamma", [1], f32, kind="ExternalInput").ap()
    out_d = nc.dram_tensor("out", [B_PER, N_SET, D], f32, kind="ExternalOutput").ap()

    with tile.TileContext(nc) as tc:
        with (
            tc.tile_pool(name="consts", bufs=1) as consts,
            tc.tile_pool(name="xp", bufs=6) as xp,
            tc.tile_pool(name="op", bufs=6) as op,
            tc.tile_pool(name="sp", bufs=2) as sp,
            tc.tile_pool(name="rp", bufs=4) as rp,
            tc.tile_pool(name="ps", bufs=2, space="PSUM") as ps,
            tc.tile_pool(name="pb", bufs=2, space="PSUM") as pb,
        ):
            # ---- constants ----
            ones_col = consts.tile([P, 1], f32)  # colsum lhsT (K=128, M=1)
            nc.vector.memset(ones_col, 1.0)
            ones_row = consts.tile([1, P], f32)
            nc.vector.memset(ones_row, 1.0)
            mu_sb = consts.tile([1, 1], f32)
            nc.sync.dma_start(mu_sb, mu_d[None, :])
            gamma_sb = consts.tile([1, 1], f32)
            nc.sync.dma_start(gamma_sb, gamma_d[None, :])
            # gamma_row[1,128] = gamma * ones (runtime scalar from SBUF)
            gamma_row = consts.tile([1, P], f32)
            nc.vector.tensor_scalar_mul(gamma_row, ones_row, gamma_sb[:])
            # mu replicated to all 128 partitions via rank-1 matmul
            psum_mu = ps.tile([P, 1], f32, tag="psmu")
            nc.tensor.matmul(
                psum_mu, lhsT=ones_row[:], rhs=mu_sb[:], start=True, stop=True
            )
            mu_col = consts.tile([P, 1], f32)
            nc.vector.tensor_copy(mu_col, psum_mu)

            # ---- per-batch pipeline ----
            for b in range(B_PER):
                x_view = x_d[b].rearrange("(p r) d -> p (r d)", p=P)
                o_view = out_d[b].rearrange("(p r) d -> p (r d)", p=P)

                xts = []
                for h in range(H):
                    xt = xp.tile([P, FH], f32, tag="xt")
                    nc.sync.dma_start(xt, x_view[:, h * FH : (h + 1) * FH])
                    xts.append(xt)

                # colsum over all 1024 set rows -> psum_s[1, 512].
                # Within-partition 4:1 r-reduce runs on the (otherwise idle)
                # GPSIMD engine; PE only reduces the partition dim of the
                # pre-summed [128, 512] tile (one matmul per half).
                psum_s = ps.tile([1, D], f32, tag="pss")
                for h in range(H):
                    xt = xts[h]
                    t0 = rp.tile([P, D], f32, tag="t0")
                    nc.gpsimd.tensor_add(t0, xt[:, 0:D], xt[:, D : 2 * D])
                    t1 = rp.tile([P, D], f32, tag="t1")
                    nc.gpsimd.tensor_add(t1, xt[:, 2 * D : 3 * D], xt[:, 3 * D : 4 * D])
                    rsum = rp.tile([P, D], f32, tag="rsum")
                    nc.gpsimd.tensor_add(rsum, t0, t1)
                    nc.tensor.matmul(
                        psum_s,
                        lhsT=ones_col[:],
                        rhs=rsum[:],
                        start=(h == 0),
                        stop=(h == H - 1),
                    )
                s_sb = sp.tile([1, D], f32, tag="ssb")
                nc.scalar.copy(s_sb, psum_s)

                # broadcast gamma*colsum to [128, 512] via rank-1 matmul
                psum_b = pb.tile([P, D], f32, tag="psb")
                nc.tensor.matmul(
                    psum_b, lhsT=gamma_row[:], rhs=s_sb[:], start=True, stop=True
                )

                # fused: out = (x * mu) + bcast   (single DVE pass per half)
                for h in range(H):
                    ot = op.tile([P, FH], f32, tag="ot")
                    nc.vector.scalar_tensor_tensor(
                        out=ot[:].rearrange("p (r d) -> p r d", r=RH),
                        in0=xts[h][:].rearrange("p (r d) -> p r d", r=RH),
                        scalar=mu_col[:],
                        in1=psum_b[:, None, :].broadcast_to([P, RH, D]),
                        op0=mybir.AluOpType.mult,
                        op1=mybir.AluOpType.add,
                    )
                    nc.scalar.dma_start(o_view[:, h * FH : (h + 1) * FH], ot)

    nc.compile()
    _cache["nc"] = nc
    return nc


def run_spmd(x, mu, gamma, **spmd_kwargs):
    nc = build_nc()
    x = np.ascontiguousarray(x, dtype=np.float32)
    mu = np.ascontiguousarray(mu, dtype=np.float32)
    gamma = np.ascontiguousarray(gamma, dtype=np.float32)
    in_maps = [
        {"x": x[c * B_PER : (c + 1) * B_PER], "mu": mu, "gamma": gamma}
        for c in range(N_CORES)
    ]
    return run_bass_kernel_spmd(nc, in_maps, list(range(N_CORES)), **spmd_kwargs)


def kernel(x, mu, gamma):
    res = run_spmd(x, mu, gamma)
    out = np.concatenate([r["out"] for r in res.results], axis=0)
    return out


# revision 12
# speedup vs baseline: 1.0901x; 1.0901x over previous
"""ContextBasedLinear Trainium2 kernel.

Computes out = mu * x + gamma * sum(x, axis=1, keepdims=True) for
x: [64, 1024, 512] f32, mu/gamma: [1] f32.

Sharding: data-parallel on the batch dim — 8 batches per core on 8
NeuronCores; mu/gamma replicated. No cross-core comms needed.

Per-core program (x_c: [8, 1024, 512]):
  Each batch's [1024, 512] lives in SBUF as [128, 4096]: partition p
  holds set rows 8p..8p+7 (16 KB contiguous per partition), processed
  in two half-tiles [128, 2048] for pipelining.
  - colsum: PE matmuls with ones[128,1] stationary reduce the
    partition dim of each 512-wide r-slice, accumulating all 8 slices
    into one PSUM row psum_s[1, 512].
  - psum_b[128,512] = (gamma ones)[1,128].T @ s[1,512]: rank-1 matmul
    broadcasts gamma * colsum to every partition.
  - out = (x * mu) + psum_b in ONE fused DVE scalar_tensor_tensor pass
    per half, with psum_b read through a step-0 broadcast AP.
  - loads issue on the SP HWDGE ring (nc.sync), stores on the ACT ring
    (nc.scalar) so store-waits can't head-of-line-block loads.
"""

import numpy as np

import concourse.bacc as bacc
import concourse.mybir as mybir
import concourse.tile as tile
from concourse.bass_utils import run_bass_kernel_spmd

N_CORES = 8
B_FULL = 64
B_PER = B_FULL // N_CORES  # 8 batches per core
N_SET = 1024
D = 512
P = 128
R = N_SET // P  # 8 set-rows per partition
F = R * D  # 4096 free elems per partition
H = 2  # half-tiles per batch
RH = R // H  # 4 r-slices per half
FH = F // H  # 2048 free elems per half

_cache = {}


def build_nc():
    if "nc" in _cache:
        return _cache["nc"]
    f32 = mybir.dt.float32
    nc = bacc.Bacc(
        "TRN2", target_bir_lowering=False, debug=False, num_devices=N_CORES
    )
    x_d = nc.dram_tensor("x", [B_PER, N_SET, D], f32, kind="ExternalInput").ap()
    mu_d = nc.dram_tensor("mu", [1], f32, kind="ExternalInput").ap()
    gamma_d = nc.dram_tensor("gamma", [1], f32, kind="ExternalInput").ap()
    out_d = nc.dram_tensor("out", [B_PER, N_SET, D], f32, kind="ExternalOutput").ap()

    with tile.TileContext(nc) as tc:
        with (
            tc.tile_pool(name="consts", bufs=1) as consts,
            tc.tile_pool(name="xp", bufs=10) as xp,
            tc.tile_pool(name="op", bufs=8) as op,
            tc.tile_pool(name="sp", bufs=2) as sp,
            tc.tile_pool(name="ps", bufs=2, space="PSUM") as ps,
            tc.tile_pool(name="pb", bufs=2, space="PSUM") as pb,
        ):
            # ---- constants ----
            ones_col = consts.tile([P, 1], f32)  # colsum lhsT (K=128, M=1)
            nc.vector.memset(ones_col, 1.0)
            ones_row = consts.tile([1, P], f32)
            nc.vector.memset(ones_row, 1.0)
            mu_sb = consts.tile([1, 1], f32)
            nc.sync.dma_start(mu_sb, mu_d[None, :])
            gamma_sb = consts.tile([1, 1], f32)
            nc.sync.dma_start(gamma_sb, gamma_d[None, :])
            # gamma_row[1,128] = gamma * ones (runtime scalar from SBUF)
            gamma_row = consts.tile([1, P], f32)
            nc.vector.tensor_scalar_mul(gamma_row, ones_row, gamma_sb[:])
            # mu replicated to all 128 partitions via rank-1 matmul
            psum_mu = ps.tile([P, 1], f32, tag="psmu")
            nc.tensor.matmul(
                psum_mu, lhsT=ones_row[:], rhs=mu_sb[:], start=True, stop=True
            )
            mu_col = consts.tile([P, 1], f32)
            nc.vector.tensor_copy(mu_col, psum_mu)

            # ---- per-batch pipeline ----
            for b in range(B_PER):
                x_view = x_d[b].rearrange("(p r) d -> p (r d)", p=P)
                o_view = out_d[b].rearrange("(p r) d -> p (r d)", p=P)

                xts = []
                for h in range(H):
                    xt = xp.tile([P, FH], f32, tag="xt")
                    nc.sync.dma_start(xt, x_view[:, h * FH : (h + 1) * FH])
                    xts.append(xt)

                # colsum over all 1024 set rows -> psum_s[1, 512]
                psum_s = ps.tile([1, D], f32, tag="pss")
                for h in range(H):
                    for j in range(RH):
                        nc.tensor.matmul(
                            psum_s,
                            lhsT=ones_col[:],
                            rhs=xts[h][:, j * D : (j + 1) * D],
                            start=(h == 0 and j == 0),
                            stop=(h == H - 1 and j == RH - 1),
                        )
                s_sb = sp.tile([1, D], f32, tag="ssb")
                nc.scalar.copy(s_sb, psum_s)

                # broadcast gamma*colsum to [128, 512] via rank-1 matmul
                psum_b = pb.tile([P, D], f32, tag="psb")
                nc.tensor.matmul(
                    psum_b, lhsT=gamma_row[:], rhs=s_sb[:], start=True, stop=True
                )

                # fused: out = (x * mu) + bcast   (single DVE pass per half)
                for h in range(H):
                    ot = op.tile([P, FH], f32, tag="ot")
                    nc.vector.scalar_tensor_tensor(
                        out=ot[:].rearrange("p (r d) -> p r d", r=RH),
                        in0=xts[h][:].rearrange("p (r d) -> p r d", r=RH),
                        scalar=mu_col[:],
                        in1=psum_b[:, None, :].broadcast_to([P, RH, D]),
                        op0=mybir.AluOpType.mult,
                        op1=mybir.AluOpType.add,
                    )
                    nc.scalar.dma_start(o_view[:, h * FH : (h + 1) * FH], ot)

    nc.compile()
    _cache["nc"] = nc
    return nc


def run_spmd(x, mu, gamma, **spmd_kwargs):
    nc = build_nc()
    x = np.ascontiguousarray(x, dtype=np.float32)
    mu = np.ascontiguousarray(mu, dtype=np.float32)
    gamma = np.ascontiguousarray(gamma, dtype=np.float32)
    in_maps = [
        {"x": x[c * B_PER : (c + 1) * B_PER], "mu": mu, "gamma": gamma}
        for c in range(N_CORES)
    ]
    return run_bass_kernel_spmd(nc, in_maps, list(range(N_CORES)), **spmd_kwargs)


def kernel(x, mu, gamma):
    res = run_spmd(x, mu, gamma)
    out = np.concatenate([r["out"] for r in res.results], axis=0)
    return out


# revision 13
# speedup vs baseline: 1.0904x; 1.0003x over previous
"""ContextBasedLinear Trainium2 kernel.

Computes out = mu * x + gamma * sum(x, axis=1, keepdims=True) for
x: [64, 1024, 512] f32, mu/gamma: [1] f32.

Sharding: data-parallel on the batch dim — 8 batches per core on 8
NeuronCores; mu/gamma replicated. No cross-core comms needed.

Per-core program (x_c: [8, 1024, 512]):
  Each batch's [1024, 512] lives in SBUF as [128, 4096]: partition p
  holds set rows 8p..8p+7 (16 KB contiguous per partition), processed
  in two half-tiles [128, 2048] for pipelining.
  - colsum: PE matmuls with ones[128,1] stationary reduce the
    partition dim of each 512-wide r-slice, accumulating all 8 slices
    into one PSUM row psum_s[1, 512].
  - psum_b[128,512] = (gamma ones)[1,128].T @ s[1,512]: rank-1 matmul
    broadcasts gamma * colsum to every partition.
  - out = (x * mu) + psum_b in ONE fused DVE scalar_tensor_tensor pass
    per half, with psum_b read through a step-0 broadcast AP.
  - loads issue on the SP HWDGE ring (nc.sync), stores on the ACT ring
    (nc.scalar) so store-waits can't head-of-line-block loads.
"""

import numpy as np

import concourse.bacc as bacc
import concourse.mybir as mybir
import concourse.tile as tile
from concourse.bass_utils import run_bass_kernel_spmd

N_CORES = 8
B_FULL = 64
B_PER = B_FULL // N_CORES  # 8 batches per core
N_SET = 1024
D = 512
P = 128
R = N_SET // P  # 8 set-rows per partition
F = R * D  # 4096 free elems per partition
H = 2  # half-tiles per batch
RH = R // H  # 4 r-slices per half
FH = F // H  # 2048 free elems per half

_cache = {}


def build_nc():
    if "nc" in _cache:
        return _cache["nc"]
    f32 = mybir.dt.float32
    nc = bacc.Bacc(
        "TRN2", target_bir_lowering=False, debug=False, num_devices=N_CORES
    )
    x_d = nc.dram_tensor("x", [B_PER, N_SET, D], f32, kind="ExternalInput").ap()
    mu_d = nc.dram_tensor("mu", [1], f32, kind="ExternalInput").ap()
    gamma_d = nc.dram_tensor("gamma", [1], f32, kind="ExternalInput").ap()
    out_d = nc.dram_tensor("out", [B_PER, N_SET, D], f32, kind="ExternalOutput").ap()

    with tile.TileContext(nc) as tc:
        with (
            tc.tile_pool(name="consts", bufs=1) as consts,
            tc.tile_pool(name="xp", bufs=12) as xp,
            tc.tile_pool(name="op", bufs=9) as op,
            tc.tile_pool(name="sp", bufs=2) as sp,
            tc.tile_pool(name="ps", bufs=2, space="PSUM") as ps,
            tc.tile_pool(name="pb", bufs=2, space="PSUM") as pb,
        ):
            # ---- constants ----
            ones_col = consts.tile([P, 1], f32)  # colsum lhsT (K=128, M=1)
            nc.vector.memset(ones_col, 1.0)
            ones_row = consts.tile([1, P], f32)
            nc.vector.memset(ones_row, 1.0)
            mu_sb = consts.tile([1, 1], f32)
            nc.sync.dma_start(mu_sb, mu_d[None, :])
            gamma_sb = consts.tile([1, 1], f32)
            nc.sync.dma_start(gamma_sb, gamma_d[None, :])
            # gamma_row[1,128] = gamma * ones (runtime scalar from SBUF)
            gamma_row = consts.tile([1, P], f32)
            nc.vector.tensor_scalar_mul(gamma_row, ones_row, gamma_sb[:])
            # mu replicated to all 128 partitions via rank-1 matmul
            psum_mu = ps.tile([P, 1], f32, tag="psmu")
            nc.tensor.matmul(
                psum_mu, lhsT=ones_row[:], rhs=mu_sb[:], start=True, stop=True
            )
            mu_col = consts.tile([P, 1], f32)
            nc.vector.tensor_copy(mu_col, psum_mu)

            # ---- per-batch pipeline ----
            for b in range(B_PER):
                x_view = x_d[b].rearrange("(p r) d -> p (r d)", p=P)
                o_view = out_d[b].rearrange("(p r) d -> p (r d)", p=P)

                xts = []
                for h in range(H):
                    xt = xp.tile([P, FH], f32, tag="xt")
                    nc.sync.dma_start(xt, x_view[:, h * FH : (h + 1) * FH])
                    xts.append(xt)

                # colsum over all 1024 set rows -> psum_s[1, 512]
                psum_s = ps.tile([1, D], f32, tag="pss")
                for h in range(H):
                    for j in range(RH):
                        nc.tensor.matmul(
                            psum_s,
                            lhsT=ones_col[:],
                            rhs=xts[h][:, j * D : (j + 1) * D],
                            start=(h == 0 and j == 0),
                            stop=(h == H - 1 and j == RH - 1),
                        )
                s_sb = sp.tile([1, D], f32, tag="ssb")
                nc.scalar.copy(s_sb, psum_s)

                # broadcast gamma*colsum to [128, 512] via rank-1 matmul
                psum_b = pb.tile([P, D], f32, tag="psb")
                nc.tensor.matmul(
                    psum_b, lhsT=gamma_row[:], rhs=s_sb[:], start=True, stop=True
                )

                # fused: out = (x * mu) + bcast   (single DVE pass per half)
                for h in range(H):
                    ot = op.tile([P, FH], f32, tag="ot")
                    nc.vector.scalar_tensor_tensor(
                        out=ot[:].rearrange("p (r d) -> p r d", r=RH),
                        in0=xts[h][:].rearrange("p (r d) -> p r d", r=RH),
                        scalar=mu_col[:],
                        in1=psum_b[:, None, :].broadcast_to([P, RH, D]),
                        op0=mybir.AluOpType.mult,
                        op1=mybir.AluOpType.add,
                    )
                    nc.scalar.dma_start(o_view[:, h * FH : (h + 1) * FH], ot)

    nc.compile()
    _cache["nc"] = nc
    return nc


def run_spmd(x, mu, gamma, **spmd_kwargs):
    nc = build_nc()
    x = np.ascontiguousarray(x, dtype=np.float32)
    mu = np.ascontiguousarray(mu, dtype=np.float32)
    gamma = np.ascontiguousarray(gamma, dtype=np.float32)
    in_maps = [
        {"x": x[c * B_PER : (c + 1) * B_PER], "mu": mu, "gamma": gamma}
        for c in range(N_CORES)
    ]
    return run_bass_kernel_spmd(nc, in_maps, list(range(N_CORES)), **spmd_kwargs)


def kernel(x, mu, gamma):
    res = run_spmd(x, mu, gamma)
    out = np.concatenate([r["out"] for r in res.results], axis=0)
    return out


# revision 15
# speedup vs baseline: 1.1012x; 1.0099x over previous
"""ContextBasedLinear Trainium2 kernel.

Computes out = mu * x + gamma * sum(x, axis=1, keepdims=True) for
x: [64, 1024, 512] f32, mu/gamma: [1] f32.

Sharding: data-parallel on the batch dim — 8 batches per core on 8
NeuronCores; mu/gamma replicated. No cross-core comms needed.

Per-core program (x_c: [8, 1024, 512]):
  Each batch's [1024, 512] lives in SBUF as [128, 4096]: partition p
  holds set rows 8p..8p+7 (16 KB contiguous per partition), processed
  in two half-tiles [128, 2048] for pipelining.
  - colsum: PE matmuls with ones[128,1] stationary reduce the
    partition dim of each 512-wide r-slice, accumulating all 8 slices
    into one PSUM row psum_s[1, 512].
  - psum_b[128,512] = (gamma ones)[1,128].T @ s[1,512]: rank-1 matmul
    broadcasts gamma * colsum to every partition.
  - out = (x * mu) + psum_b in ONE fused DVE scalar_tensor_tensor pass
    per half, with psum_b read through a step-0 broadcast AP.
  - loads issue on the SP HWDGE ring (nc.sync), stores on the ACT ring
    (nc.scalar) so store-waits can't head-of-line-block loads.
"""

import numpy as np

import concourse.bacc as bacc
import concourse.mybir as mybir
import concourse.tile as tile
from concourse.bass_utils import run_bass_kernel_spmd

N_CORES = 8
B_FULL = 64
B_PER = B_FULL // N_CORES  # 8 batches per core
N_SET = 1024
D = 512
P = 128
R = N_SET // P  # 8 set-rows per partition
F = R * D  # 4096 free elems per partition
H = 2  # half-tiles per batch
RH = R // H  # 4 r-slices per half
FH = F // H  # 2048 free elems per half

_cache = {}


def build_nc():
    if "nc" in _cache:
        return _cache["nc"]
    f32 = mybir.dt.float32
    nc = bacc.Bacc(
        "TRN2", target_bir_lowering=False, debug=False, num_devices=N_CORES
    )
    x_d = nc.dram_tensor("x", [B_PER, N_SET, D], f32, kind="ExternalInput").ap()
    mu_d = nc.dram_tensor("mu", [1], f32, kind="ExternalInput").ap()
    gamma_d = nc.dram_tensor("gamma", [1], f32, kind="ExternalInput").ap()
    out_d = nc.dram_tensor("out", [B_PER, N_SET, D], f32, kind="ExternalOutput").ap()

    with tile.TileContext(nc) as tc:
        with (
            tc.tile_pool(name="consts", bufs=1) as consts,
            tc.tile_pool(name="xp", bufs=12) as xp,
            tc.tile_pool(name="op", bufs=9) as op,
            tc.tile_pool(name="sp", bufs=2) as sp,
            tc.tile_pool(name="ps", bufs=2, space="PSUM") as ps,
            tc.tile_pool(name="pb", bufs=2, space="PSUM") as pb,
        ):
            # ---- constants ----
            ones_col = consts.tile([P, 1], f32)  # colsum lhsT (K=128, M=1)
            nc.vector.memset(ones_col, 1.0)
            ones_row = consts.tile([1, P], f32)
            nc.vector.memset(ones_row, 1.0)
            # mu/gamma ride the (head-idle) ACT ring: each 4 B HBM DMA pays
            # a ~2.4 us completion round-trip, and on the sync ring the two
            # of them would serialize ahead of the first 1 MB x load.
            mu_sb = consts.tile([1, 1], f32)
            nc.scalar.dma_start(mu_sb, mu_d[None, :])
            gamma_sb = consts.tile([1, 1], f32)
            nc.scalar.dma_start(gamma_sb, gamma_d[None, :])
            # gamma_row[1,128] = gamma * ones (runtime scalar from SBUF)
            gamma_row = consts.tile([1, P], f32)
            nc.vector.tensor_scalar_mul(gamma_row, ones_row, gamma_sb[:])
            # mu replicated to all 128 partitions via rank-1 matmul
            psum_mu = ps.tile([P, 1], f32, tag="psmu")
            nc.tensor.matmul(
                psum_mu, lhsT=ones_row[:], rhs=mu_sb[:], start=True, stop=True
            )
            mu_col = consts.tile([P, 1], f32)
            nc.vector.tensor_copy(mu_col, psum_mu)

            # ---- per-batch pipeline ----
            for b in range(B_PER):
                x_view = x_d[b].rearrange("(p r) d -> p (r d)", p=P)
                o_view = out_d[b].rearrange("(p r) d -> p (r d)", p=P)

                xts = []
                for h in range(H):
                    xt = xp.tile([P, FH], f32, tag="xt")
                    nc.sync.dma_start(xt, x_view[:, h * FH : (h + 1) * FH])
                    xts.append(xt)

                # colsum over all 1024 set rows -> psum_s[1, 512]
                psum_s = ps.tile([1, D], f32, tag="pss")
                for h in range(H):
                    for j in range(RH):
                        nc.tensor.matmul(
                            psum_s,
                            lhsT=ones_col[:],
                            rhs=xts[h][:, j * D : (j + 1) * D],
                            start=(h == 0 and j == 0),
                            stop=(h == H - 1 and j == RH - 1),
                        )
                s_sb = sp.tile([1, D], f32, tag="ssb")
                nc.scalar.copy(s_sb, psum_s)

                # broadcast gamma*colsum to [128, 512] via rank-1 matmul
                psum_b = pb.tile([P, D], f32, tag="psb")
                nc.tensor.matmul(
                    psum_b, lhsT=gamma_row[:], rhs=s_sb[:], start=True, stop=True
                )

                # fused: out = (x * mu) + bcast   (single DVE pass per chunk).
                # The last batch runs quarter-size chunks so the kernel tail
                # (final STT + final store) is half as long.
                nq = 2 if b == B_PER - 1 else 1
                fq = FH // nq
                rq = RH // nq
                for h in range(H):
                    for q in range(nq):
                        ot = op.tile([P, fq], f32, tag="ot")
                        nc.vector.scalar_tensor_tensor(
                            out=ot[:].rearrange("p (r d) -> p r d", r=rq),
                            in0=xts[h][:, q * fq : (q + 1) * fq].rearrange(
                                "p (r d) -> p r d", r=rq
                            ),
                            scalar=mu_col[:],
                            in1=psum_b[:, None, :].broadcast_to([P, rq, D]),
                            op0=mybir.AluOpType.mult,
                            op1=mybir.AluOpType.add,
                        )
                        nc.scalar.dma_start(
                            o_view[:, h * FH + q * fq : h * FH + (q + 1) * fq], ot
                        )

    nc.compile()
    _cache["nc"] = nc
    return nc


def run_spmd(x, mu, gamma, **spmd_kwargs):
    nc = build_nc()
    x = np.ascontiguousarray(x, dtype=np.float32)
    mu = np.ascontiguousarray(mu, dtype=np.float32)
    gamma = np.ascontiguousarray(gamma, dtype=np.float32)
    in_maps = [
        {"x": x[c * B_PER : (c + 1) * B_PER], "mu": mu, "gamma": gamma}
        for c in range(N_CORES)
    ]
    return run_bass_kernel_spmd(nc, in_maps, list(range(N_CORES)), **spmd_kwargs)


def kernel(x, mu, gamma):
    res = run_spmd(x, mu, gamma)
    out = np.concatenate([r["out"] for r in res.results], axis=0)
    return out


# revision 18
# speedup vs baseline: 1.1261x; 1.0226x over previous
"""ContextBasedLinear Trainium2 kernel.

Computes out = mu * x + gamma * sum(x, axis=1, keepdims=True) for
x: [64, 1024, 512] f32, mu/gamma: [1] f32.

Sharding: data-parallel on the batch dim — 8 batches per core on 8
NeuronCores; mu/gamma replicated. No cross-core comms needed.

Per-core program (x_c: [8, 1024, 512]):
  Each batch's [1024, 512] lives in SBUF as [128, 4096]: partition p
  holds set rows 8p..8p+7 (16 KB contiguous per partition), processed
  in two half-tiles [128, 2048] for pipelining.
  - colsum: PE matmuls with ones[128,1] stationary reduce the
    partition dim of each 512-wide r-slice, accumulating all 8 slices
    into one PSUM row psum_s[1, 512].
  - psum_b[128,512] = (gamma ones)[1,128].T @ s[1,512]: rank-1 matmul
    broadcasts gamma * colsum to every partition.
  - out = (x * mu) + psum_b in ONE fused DVE scalar_tensor_tensor pass
    per half, with psum_b read through a step-0 broadcast AP.
  - loads issue on the SP HWDGE ring (nc.sync), stores on the ACT ring
    (nc.scalar) so store-waits can't head-of-line-block loads.
"""

import numpy as np

import concourse.bacc as bacc
import concourse.mybir as mybir
import concourse.tile as tile
from concourse.bass_utils import run_bass_kernel_spmd

N_CORES = 8
B_FULL = 64
B_PER = B_FULL // N_CORES  # 8 batches per core
N_SET = 1024
D = 512
P = 128
R = N_SET // P  # 8 set-rows per partition
F = R * D  # 4096 free elems per partition
H = 2  # half-tiles per batch
RH = R // H  # 4 r-slices per half
FH = F // H  # 2048 free elems per half

_cache = {}


def build_nc():
    if "nc" in _cache:
        return _cache["nc"]
    f32 = mybir.dt.float32
    nc = bacc.Bacc(
        "TRN2", target_bir_lowering=False, debug=False, num_devices=N_CORES
    )
    x_d = nc.dram_tensor("x", [B_PER, N_SET, D], f32, kind="ExternalInput").ap()
    mu_d = nc.dram_tensor("mu", [1], f32, kind="ExternalInput").ap()
    gamma_d = nc.dram_tensor("gamma", [1], f32, kind="ExternalInput").ap()
    out_d = nc.dram_tensor("out", [B_PER, N_SET, D], f32, kind="ExternalOutput").ap()

    with tile.TileContext(nc) as tc:
        with (
            tc.tile_pool(name="consts", bufs=1) as consts,
            tc.tile_pool(name="xp", bufs=6) as xp,
            tc.tile_pool(name="op", bufs=9) as op,
            tc.tile_pool(name="sp", bufs=2) as sp,
            tc.tile_pool(name="ps", bufs=2, space="PSUM") as ps,
            tc.tile_pool(name="pb", bufs=2, space="PSUM") as pb,
        ):
            # ---- constants ----
            ones_col = consts.tile([P, 1], f32)  # colsum lhsT (K=128, M=1)
            nc.vector.memset(ones_col, 1.0)
            ones_row = consts.tile([1, P], f32)
            nc.vector.memset(ones_row, 1.0)
            # mu/gamma ride the (head-idle) ACT ring: each 4 B HBM DMA pays
            # a ~2.4 us completion round-trip, and on the sync ring the two
            # of them would serialize ahead of the first 1 MB x load.
            mu_sb = consts.tile([1, 1], f32)
            nc.scalar.dma_start(mu_sb, mu_d[None, :])
            gamma_sb = consts.tile([1, 1], f32)
            nc.scalar.dma_start(gamma_sb, gamma_d[None, :])
            # gamma_row[1,128] = gamma * ones (runtime scalar from SBUF)
            gamma_row = consts.tile([1, P], f32)
            nc.vector.tensor_scalar_mul(gamma_row, ones_row, gamma_sb[:])
            # mu replicated to all 128 partitions via rank-1 matmul
            psum_mu = ps.tile([P, 1], f32, tag="psmu")
            nc.tensor.matmul(
                psum_mu, lhsT=ones_row[:], rhs=mu_sb[:], start=True, stop=True
            )
            mu_col = consts.tile([P, 1], f32)
            nc.vector.tensor_copy(mu_col, psum_mu)

            # ---- per-batch pipeline ----
            for b in range(B_PER):
                x_view = x_d[b].rearrange("(p r) d -> p (r d)", p=P)
                o_view = out_d[b].rearrange("(p r) d -> p (r d)", p=P)

                # Steady-state batches load as ONE 2 MB transfer (better
                # DMA efficiency); the last batch keeps 1 MB half loads so
                # its compute can start before the full tile lands.
                last = b == B_PER - 1
                if last:
                    xts = []
                    for h in range(H):
                        xt = xp.tile([P, FH], f32, tag="xt")
                        nc.sync.dma_start(xt, x_view[:, h * FH : (h + 1) * FH])
                        xts.append(xt)
                    halves = [(xts[h], 0) for h in range(H)]
                else:
                    xt = xp.tile([P, F], f32, tag="xt")
                    nc.sync.dma_start(xt, x_view)
                    halves = [(xt, h * FH) for h in range(H)]

                # colsum over all 1024 set rows -> psum_s[1, 512]
                psum_s = ps.tile([1, D], f32, tag="pss")
                for h, (xth, off) in enumerate(halves):
                    for j in range(RH):
                        nc.tensor.matmul(
                            psum_s,
                            lhsT=ones_col[:],
                            rhs=xth[:, off + j * D : off + (j + 1) * D],
                            start=(h == 0 and j == 0),
                            stop=(h == H - 1 and j == RH - 1),
                        )
                s_sb = sp.tile([1, D], f32, tag="ssb")
                nc.scalar.copy(s_sb, psum_s)

                # broadcast gamma*colsum to [128, 512] via rank-1 matmul
                psum_b = pb.tile([P, D], f32, tag="psb")
                nc.tensor.matmul(
                    psum_b, lhsT=gamma_row[:], rhs=s_sb[:], start=True, stop=True
                )

                # fused: out = (x * mu) + bcast   (single DVE pass per chunk).
                # The last batch runs quarter-size chunks so the kernel tail
                # (final STT + final store) is half as long.
                nq = 2 if last else 1
                fq = FH // nq
                rq = RH // nq
                for h, (xth, off) in enumerate(halves):
                    for q in range(nq):
                        ot = op.tile([P, fq], f32, tag="ot")
                        nc.vector.scalar_tensor_tensor(
                            out=ot[:].rearrange("p (r d) -> p r d", r=rq),
                            in0=xth[:, off + q * fq : off + (q + 1) * fq].rearrange(
                                "p (r d) -> p r d", r=rq
                            ),
                            scalar=mu_col[:],
                            in1=psum_b[:, None, :].broadcast_to([P, rq, D]),
                            op0=mybir.AluOpType.mult,
                            op1=mybir.AluOpType.add,
                        )
                        nc.scalar.dma_start(
                            o_view[:, h * FH + q * fq : h * FH + (q + 1) * fq], ot
                        )

    nc.compile()
    _cache["nc"] = nc
    return nc


def run_spmd(x, mu, gamma, **spmd_kwargs):
    nc = build_nc()
    x = np.ascontiguousarray(x, dtype=np.float32)
    mu = np.ascontiguousarray(mu, dtype=np.float32)
    gamma = np.ascontiguousarray(gamma, dtype=np.float32)
    in_maps = [
        {"x": x[c * B_PER : (c + 1) * B_PER], "mu": mu, "gamma": gamma}
        for c in range(N_CORES)
    ]
    return run_bass_kernel_spmd(nc, in_maps, list(range(N_CORES)), **spmd_kwargs)


def kernel(x, mu, gamma):
    res = run_spmd(x, mu, gamma)
    out = np.concatenate([r["out"] for r in res.results], axis=0)
    return out
